# revision 1
# baseline (speedup 1.0000x reference)
"""Multi-head attention (B=2, S=2048, E=1024, H=16) on 8 Trainium2 NeuronCores.

Sharding: core c handles batch b=c//4 and head group g=c%4 (4 heads each).
hidden_states[b] is replicated to the 4 cores of batch b (pre-transposed and
cast to bf16 on host so the contraction dim E lands on SBUF partitions with
plain contiguous DMAs). Each core computes q/k/v projections for its heads,
transposed-layout attention (scores^T = k q'^T so softmax reduces over the
PSUM partition dim via a ones-matmul), and a partial output projection over
its 256 E-dims. The host sums the 4 partials per batch and adds bo.

Bias handling: softmax over t is invariant to per-query constants, so the
k-bias drops out entirely and the q-bias is folded into q' = q + bq. The
v-bias is a post-softmax additive constant (softmax rows sum to 1), applied
after normalization. bo is added on host.
"""

import sys

if "/opt/trn_rl_repo" not in sys.path:
    sys.path.insert(0, "/opt/trn_rl_repo")

import numpy as np
import ml_dtypes

import concourse.bass as bass
import concourse.tile as tile
from concourse import mybir
from concourse.bass_utils import run_bass_kernel_spmd
from concourse.vector_clock import ScopedClock

B, S, E, H = 2, 2048, 1024, 16
DH = E // H  # 64
N_CORES = 8
HEADS_PER_CORE = 4  # 2 pairs
EL = HEADS_PER_CORE * DH  # 256 local E-dims per core

F32 = mybir.dt.float32
BF16 = mybir.dt.bfloat16
BF16_NP = ml_dtypes.bfloat16

ST = 512  # s_tile width (softmax free dim per psum bank)
N_ST = S // ST  # 4
N_TC = S // 128  # 16 t-chunks
N_EC = E // 128  # 8 e-chunks


def _patch_tail_drain():
    """walrus CoreV3 setupSyncWait allows only 1 sem wait on an SP Drain; Tile's
    kernel-tail drain carries one wait per live processor. Split the waits
    across consecutive drains (mutating via nc.inst_map, whose objects are what
    to_json_bytes serializes)."""
    if getattr(tile.TileContext, "_drain_patched", False):
        return

    def _drain_and_barrier(self, tick_clock, wait_clock):
        nc = self.nc
        drain_inst = nc.sync.drain()
        wait_clock.add_sem_waits(
            drain_inst.ins, ScopedClock({None: tick_clock.global_clock})
        )
        inst = nc.inst_map[drain_inst.ins.name]
        w = list(inst.sync_info.on_wait) if inst.sync_info else []
        if len(w) > 1:
            si = inst.sync_info
            si.on_wait = w[:1]
            inst.sync_info = si
            for i in range(1, len(w)):
                d2 = nc.sync.drain()
                i2 = nc.inst_map[d2.ins.name]
                si2 = i2.sync_info or mybir.SyncInfo(on_wait=[], on_update=[])
                si2.on_wait = [w[i]]
                i2.sync_info = si2
        nc.all_engine_barrier()
        assert self.sems is not None
        popped = nc._tile_sem_poison_stack.pop()
        assert popped is self._sem_poison
        nc.clear_and_free_semaphores(list(self.sems.allocated().values()))
        nc.all_engine_barrier()

    tile.TileContext._drain_and_barrier = _drain_and_barrier
    tile.TileContext._drain_patched = True


def _split_multi_waits(nc):
    """The walrus build in this environment accepts only ONE sem-wait command
    per instruction, but Tile's wait-assignment attaches several. Hoist excess
    waits onto dedicated same-engine no-op carrier instructions inserted
    immediately before the owner (same engine-stream position, identical
    semantics)."""
    f = nc.m.functions[0]
    blocks = list(f.blocks)
    carriers: dict[str, list] = {}
    created = set()
    for blk in blocks:
        for inst in blk.instructions:
            if inst.sync_info and len(inst.sync_info.on_wait) > 1:
                w = list(inst.sync_info.on_wait)
                cs = []
                for wx in w[:-1]:
                    # engine nop() appends to nc.cur_bb; it is re-homed below
                    nop = nc.engines[inst.engine].nop(nofuse=True).ins
                    nop.sync_info = mybir.SyncInfo(on_wait=[wx], on_update=[])
                    cs.append(nop)
                    created.add(nop.name)
                si = inst.sync_info
                si.on_wait = [w[-1]]
                inst.sync_info = si
                carriers[inst.name] = cs
    if not carriers:
        return
    for blk in blocks:
        rebuilt = []
        for i in blk.instructions:
            if i.name in created:
                continue
            rebuilt.extend(carriers.get(i.name, ()))
            rebuilt.append(i)
        blk.instructions = rebuilt


def build_bass():
    """Build the per-core Bass program (identical on all 8 cores)."""
    _patch_tail_drain()
    nc = bass.Bass("TRN2", target_bir_lowering=False, debug=False)

    xt_d = nc.dram_tensor("xt", [E, S], BF16, kind="ExternalInput").ap()
    wq_d = nc.dram_tensor("wq", [E, EL], BF16, kind="ExternalInput").ap()
    wk_d = nc.dram_tensor("wk", [E, EL], BF16, kind="ExternalInput").ap()
    wv_d = nc.dram_tensor("wv", [E, EL], BF16, kind="ExternalInput").ap()
    wo_d = nc.dram_tensor("wo", [EL, E], BF16, kind="ExternalInput").ap()
    bq_d = nc.dram_tensor("bq2", [128, 2], F32, kind="ExternalInput").ap()
    bv_d = nc.dram_tensor("bv2", [128, 2], F32, kind="ExternalInput").ap()
    out_d = nc.dram_tensor("out", [S, E], F32, kind="ExternalOutput").ap()

    EXP = mybir.ActivationFunctionType.Exp
    ADD = mybir.AluOpType.add
    MULT = mybir.AluOpType.mult

    with tile.TileContext(nc) as tc:
        with (
            tc.tile_pool(name="const", bufs=1) as const_pool,
            tc.tile_pool(name="xw", bufs=1) as xw_pool,
            tc.tile_pool(name="qkv", bufs=1) as qkv_pool,
            tc.tile_pool(name="exps", bufs=3) as exp_pool,
            tc.tile_pool(name="ctxn", bufs=4) as ctxn_pool,
            tc.tile_pool(name="small", bufs=4) as small_pool,
            tc.tile_pool(name="rb", bufs=4) as rb_pool,
            tc.tile_pool(name="outs", bufs=3) as out_pool,
            tc.tile_pool(name="pp", bufs=2, space="PSUM") as pp_ps,
            tc.tile_pool(name="sc", bufs=2, space="PSUM") as sc_ps,
            tc.tile_pool(name="ctx", bufs=1, space="PSUM") as ctx_ps_pool,
            tc.tile_pool(name="den", bufs=1, space="PSUM") as den_ps_pool,
        ):
            # ---- constants and weights
            ones_sb = const_pool.tile([128, 1], BF16)
            nc.vector.memset(ones_sb[:], 1.0)
            ones1_sb = const_pool.tile([1, 64], mybir.dt.float16)
            nc.vector.memset(ones1_sb[:], 1.0)
            bq_sb = const_pool.tile([128, 2], F32)
            nc.sync.dma_start(bq_sb[:], bq_d[:])
            bv_sb = const_pool.tile([128, 2], F32)
            nc.sync.dma_start(bv_sb[:], bv_d[:])

            wq_sb = xw_pool.tile([128, N_EC, EL], BF16)
            nc.sync.dma_start(wq_sb[:], wq_d.rearrange("(o p) d -> p o d", p=128))
            wk_sb = xw_pool.tile([128, N_EC, EL], BF16)
            nc.sync.dma_start(wk_sb[:], wk_d.rearrange("(o p) d -> p o d", p=128))
            wv_sb = xw_pool.tile([128, N_EC, EL], BF16)
            nc.sync.dma_start(wv_sb[:], wv_d.rearrange("(o p) d -> p o d", p=128))
            wo_sb = xw_pool.tile([128, 2, E], BF16)
            nc.sync.dma_start(wo_sb[:], wo_d.rearrange("(o p) n -> p o n", p=128))

            xt_sb = xw_pool.tile([128, N_EC, S], BF16)
            for ec in range(N_EC):
                nc.sync.dma_start(xt_sb[:, ec, :], xt_d[128 * ec : 128 * (ec + 1), :])

            # ---- projections: q'^T (with bias), k^T, v (natural layout)
            qT = [qkv_pool.tile([128, S], BF16, name=f"qT{p}") for p in range(2)]
            kT = [qkv_pool.tile([128, S], BF16, name=f"kT{p}") for p in range(2)]
            v_sb = qkv_pool.tile([128, N_TC, EL], BF16)

            for p in range(2):
                dlo, dhi = 128 * p, 128 * (p + 1)
                for st in range(N_ST):
                    slo, shi = ST * st, ST * (st + 1)
                    ps_q = pp_ps.tile([128, ST], F32, tag="pp")
                    for ec in range(N_EC):
                        nc.tensor.matmul(
                            ps_q[:],
                            wq_sb[:, ec, dlo:dhi],
                            xt_sb[:, ec, slo:shi],
                            start=(ec == 0),
                            stop=(ec == N_EC - 1),
                        )
                    nc.vector.tensor_scalar(
                        qT[p][:, slo:shi], ps_q[:], bq_sb[:, p : p + 1], None, ADD
                    )
                    ps_k = pp_ps.tile([128, ST], F32, tag="pp")
                    for ec in range(N_EC):
                        nc.tensor.matmul(
                            ps_k[:],
                            wk_sb[:, ec, dlo:dhi],
                            xt_sb[:, ec, slo:shi],
                            start=(ec == 0),
                            stop=(ec == N_EC - 1),
                        )
                    nc.vector.tensor_copy(kT[p][:, slo:shi], ps_k[:])

            for tt in range(N_TC):
                ps_v = pp_ps.tile([128, ST], F32, tag="pp")
                for ec in range(N_EC):
                    nc.tensor.matmul(
                        ps_v[:, :EL],
                        xt_sb[:, ec, 128 * tt : 128 * (tt + 1)],
                        wv_sb[:, ec, :],
                        start=(ec == 0),
                        stop=(ec == N_EC - 1),
                    )
                nc.vector.tensor_copy(v_sb[:, tt, :], ps_v[:, :EL])

            # ---- attention + output projection, per s_tile
            for st in range(N_ST):
                slo, shi = ST * st, ST * (st + 1)
                cns = []
                for p in range(2):
                    ctx_ps = ctx_ps_pool.tile([128, ST], F32)
                    den_ps = den_ps_pool.tile([128, ST], F32)
                    for tc in range(N_TC):
                        tlo, thi = 128 * tc, 128 * (tc + 1)
                        sc = sc_ps.tile([128, 2 * ST], F32)
                        nc.tensor.matmul(
                            sc[:, :ST],
                            kT[p][0:64, tlo:thi],
                            qT[p][0:64, slo:shi],
                            start=True,
                            stop=True,
                        )
                        nc.tensor.matmul(
                            sc[:, ST:],
                            kT[p][64:128, tlo:thi],
                            qT[p][64:128, slo:shi],
                            start=True,
                            stop=True,
                        )
                        ex = exp_pool.tile([128, 2 * ST], BF16)
                        nc.scalar.activation(ex[:], sc[:], EXP, scale=0.125)
                        first, last = tc == 0, tc == N_TC - 1
                        nc.tensor.matmul(
                            ctx_ps[0:64, :],
                            v_sb[:, tc, 128 * p : 128 * p + 64],
                            ex[:, :ST],
                            start=first,
                            stop=last,
                        )
                        nc.tensor.matmul(
                            ctx_ps[64:128, :],
                            v_sb[:, tc, 128 * p + 64 : 128 * (p + 1)],
                            ex[:, ST:],
                            start=first,
                            stop=last,
                        )
                        nc.tensor.matmul(
                            den_ps[0:1, :], ones_sb[:], ex[:, :ST],
                            start=first, stop=last,
                        )
                        nc.tensor.matmul(
                            den_ps[64:65, :], ones_sb[:], ex[:, ST:],
                            start=first, stop=last,
                        )
                    # normalize: ctx / denom + bv  (denom recip rows broadcast
                    # across the 64 head partitions via a stride-0 DMA read)
                    r0 = small_pool.tile([1, ST], mybir.dt.float16, tag="r0")
                    r1 = small_pool.tile([1, ST], mybir.dt.float16, tag="r1")
                    with nc.allow_low_precision(
                        reason="fp16 reciprocal rows: 5e-4 rel err, well under bf16 ctx"
                    ):
                        nc.vector.reciprocal(r0[:], den_ps[0:1, :])
                        nc.vector.reciprocal(r1[:], den_ps[64:65, :])
                    rbp = pp_ps.tile([128, ST], F32, tag="pp")
                    nc.tensor.matmul(
                        rbp[0:64, :],
                        ones1_sb[:],
                        r0[:],
                        start=True,
                        stop=True,
                    )
                    nc.tensor.matmul(
                        rbp[64:128, :],
                        ones1_sb[:],
                        r1[:],
                        start=True,
                        stop=True,
                    )
                    rb = rb_pool.tile([128, ST], F32, tag="rb")
                    nc.vector.tensor_copy(rb[:], rbp[:])
                    cn = ctxn_pool.tile([128, ST], BF16)
                    nc.vector.tensor_tensor(
                        cn[0:64, :], ctx_ps[0:64, :], rb[0:64, :], MULT
                    )
                    nc.vector.tensor_tensor(
                        cn[64:128, :], ctx_ps[64:128, :], rb[64:128, :], MULT
                    )
                    nc.vector.tensor_scalar(
                        cn[:], cn[:], bv_sb[:, p : p + 1], None, ADD
                    )
                    cns.append(cn)
                # output projection for this s_tile
                for ss in range(ST // 128):
                    srow = slo + 128 * ss
                    for nt in range(E // ST):
                        ps_o = pp_ps.tile([128, ST], F32, tag="pp")
                        nc.tensor.matmul(
                            ps_o[:],
                            cns[0][:, 128 * ss : 128 * (ss + 1)],
                            wo_sb[:, 0, ST * nt : ST * (nt + 1)],
                            start=True,
                            stop=False,
                        )
                        nc.tensor.matmul(
                            ps_o[:],
                            cns[1][:, 128 * ss : 128 * (ss + 1)],
                            wo_sb[:, 1, ST * nt : ST * (nt + 1)],
                            start=False,
                            stop=True,
                        )
                        ob = out_pool.tile([128, ST], F32)
                        nc.vector.tensor_copy(ob[:], ps_o[:])
                        nc.sync.dma_start(
                            out_d[srow : srow + 128, ST * nt : ST * (nt + 1)], ob[:]
                        )
    _split_multi_waits(nc)
    return nc


_NC = None


def _get_nc():
    global _NC
    if _NC is None:
        _NC = build_bass()
    return _NC


def make_in_maps(hidden_states, Wq, bq, Wk, bk, Wv, bv, Wo):
    """Host-side sharding/layout prep. Returns list of 8 per-core input dicts."""
    hs = np.asarray(hidden_states, dtype=np.float32)
    Wq = np.asarray(Wq, dtype=np.float32)
    Wk = np.asarray(Wk, dtype=np.float32)
    Wv = np.asarray(Wv, dtype=np.float32)
    Wo = np.asarray(Wo, dtype=np.float32)
    bq = np.asarray(bq, dtype=np.float32)
    bv = np.asarray(bv, dtype=np.float32)

    xt = [
        np.ascontiguousarray(hs[b].T).astype(BF16_NP) for b in range(B)
    ]  # [E, S] bf16
    in_maps = []
    for c in range(N_CORES):
        b, g = divmod(c, N_CORES // B)
        h0 = HEADS_PER_CORE * g
        hsl = slice(h0, h0 + HEADS_PER_CORE)
        # [H_loc, E, DH] -> [E, H_loc*DH] head-major columns
        wq_c = np.ascontiguousarray(
            Wq[hsl].transpose(1, 0, 2).reshape(E, EL)
        ).astype(BF16_NP)
        wk_c = np.ascontiguousarray(
            Wk[hsl].transpose(1, 0, 2).reshape(E, EL)
        ).astype(BF16_NP)
        wv_c = np.ascontiguousarray(
            Wv[hsl].transpose(1, 0, 2).reshape(E, EL)
        ).astype(BF16_NP)
        wo_c = np.ascontiguousarray(Wo[EL * g : EL * (g + 1), :]).astype(BF16_NP)
        bq_c = np.ascontiguousarray(bq[hsl].reshape(EL).reshape(2, 128).T)
        bv_c = np.ascontiguousarray(bv[hsl].reshape(EL).reshape(2, 128).T)
        in_maps.append(
            {
                "xt": xt[b],
                "wq": wq_c,
                "wk": wk_c,
                "wv": wv_c,
                "wo": wo_c,
                "bq2": bq_c,
                "bv2": bv_c,
            }
        )
    return in_maps


def kernel(hidden_states, mask, Wq, bq, Wk, bk, Wv, bv, Wo, bo, **run_kwargs):
    """Full-input entry point. mask is all-ones per the problem spec (ignored)."""
    nc = _get_nc()
    in_maps = make_in_maps(hidden_states, Wq, bq, Wk, bk, Wv, bv, Wo)
    res = run_bass_kernel_spmd(nc, in_maps, core_ids=list(range(N_CORES)), **run_kwargs)
    bo = np.asarray(bo, dtype=np.float32)
    out = np.zeros((B, S, E), dtype=np.float32)
    for c in range(N_CORES):
        out[c // (N_CORES // B)] += res.results[c]["out"]
    out += bo
    kernel.last_results = res
    return out



# revision 6
# speedup vs baseline: 1.1447x; 1.1447x over previous
"""Multi-head attention (B=2, S=2048, E=1024, H=16) on 8 Trainium2 NeuronCores.

Sharding: core c handles batch b=c//4 and head group g=c%4 (4 heads each).
hidden_states[b] is replicated to the 4 cores of batch b (pre-transposed and
cast to bf16 on host so the contraction dim E lands on SBUF partitions with
plain contiguous DMAs). Each core computes q/k/v projections for its heads,
transposed-layout attention (scores^T = k q'^T so softmax reduces over the
PSUM partition dim), and a partial output projection over its 256 E-dims.
The host sums the 4 partials per batch and adds bo'.

Softmax denominator: each head's v lhsT carries an extra all-ones column, so
the ctx matmul accumulates ctx rows 0:64 AND the denominator at row 64 of the
same PSUM bank for free. The four per-head denominator rows of an s-tile are
gathered to partitions {0,32,64,96} of one tile, reciprocated in ONE vector op
(recip is ~6.5 ns/col regardless of partition count, so batching partitions is
4x cheaper), broadcast to 64 partitions via tiny K=1 ones-matmuls, and applied
with tensor_tensor.

Bias handling: softmax over t is invariant to per-query constants, so the
k-bias drops out and the q-bias is folded into q' = q + bq. The v-bias is a
post-softmax additive constant (softmax rows sum to 1) that commutes with the
output projection: host adds bo' = bo + concat(bv) @ Wo, so bv never touches
the device.

Emission order keeps the Scalar engine (exp is irreducibly ~142us there, the
kernel's critical resource) fed as early and continuously as possible: qT(st0)
and kT chunk 0 project first, then st0's attention runs with the v-projection
and remaining kT chunks interleaved into its tc loop; later qT tiles project
inside the st boundaries.
"""

import sys

if "/opt/trn_rl_repo" not in sys.path:
    sys.path.insert(0, "/opt/trn_rl_repo")

import numpy as np
import ml_dtypes

import concourse.bass as bass
import concourse.tile as tile
from concourse import mybir
from concourse.bass_utils import run_bass_kernel_spmd
from concourse.vector_clock import ScopedClock

B, S, E, H = 2, 2048, 1024, 16
DH = E // H  # 64
N_CORES = 8
HEADS_PER_CORE = 4
EL = HEADS_PER_CORE * DH  # 256 local E-dims per core

F32 = mybir.dt.float32
BF16 = mybir.dt.bfloat16
FP16 = mybir.dt.float16
BF16_NP = ml_dtypes.bfloat16

ST = 512  # s_tile width
N_ST = S // ST  # 4
N_TC = S // 128  # 16 t-chunks
N_EC = E // 128  # 8 e-chunks


def _patch_tail_drain():
    """walrus CoreV3 setupSyncWait allows only 1 sem wait on an SP Drain; Tile's
    kernel-tail drain carries one wait per live processor. Split the waits
    across consecutive drains (mutating via nc.inst_map, whose objects are what
    to_json_bytes serializes)."""
    if getattr(tile.TileContext, "_drain_patched", False):
        return

    def _drain_and_barrier(self, tick_clock, wait_clock):
        nc = self.nc
        drain_inst = nc.sync.drain()
        wait_clock.add_sem_waits(
            drain_inst.ins, ScopedClock({None: tick_clock.global_clock})
        )
        inst = nc.inst_map[drain_inst.ins.name]
        w = list(inst.sync_info.on_wait) if inst.sync_info else []
        if len(w) > 1:
            si = inst.sync_info
            si.on_wait = w[:1]
            inst.sync_info = si
            for i in range(1, len(w)):
                d2 = nc.sync.drain()
                i2 = nc.inst_map[d2.ins.name]
                si2 = i2.sync_info or mybir.SyncInfo(on_wait=[], on_update=[])
                si2.on_wait = [w[i]]
                i2.sync_info = si2
        nc.all_engine_barrier()
        assert self.sems is not None
        popped = nc._tile_sem_poison_stack.pop()
        assert popped is self._sem_poison
        nc.clear_and_free_semaphores(list(self.sems.allocated().values()))
        nc.all_engine_barrier()

    tile.TileContext._drain_and_barrier = _drain_and_barrier
    tile.TileContext._drain_patched = True


def _split_multi_waits(nc):
    """The walrus build in this environment accepts only ONE sem-wait command
    per instruction, but Tile's wait-assignment attaches several. Hoist excess
    waits onto dedicated same-engine no-op carrier instructions inserted
    immediately before the owner (same engine-stream position, identical
    semantics)."""
    f = nc.m.functions[0]
    blocks = list(f.blocks)
    carriers: dict[str, list] = {}
    created = set()
    for blk in blocks:
        for inst in blk.instructions:
            if inst.sync_info and len(inst.sync_info.on_wait) > 1:
                w = list(inst.sync_info.on_wait)
                cs = []
                for wx in w[:-1]:
                    # engine nop() appends to nc.cur_bb; it is re-homed below
                    nop = nc.engines[inst.engine].nop(nofuse=True).ins
                    nop.sync_info = mybir.SyncInfo(on_wait=[wx], on_update=[])
                    cs.append(nop)
                    created.add(nop.name)
                si = inst.sync_info
                si.on_wait = [w[-1]]
                inst.sync_info = si
                carriers[inst.name] = cs
    if not carriers:
        return
    for blk in blocks:
        rebuilt = []
        for i in blk.instructions:
            if i.name in created:
                continue
            rebuilt.extend(carriers.get(i.name, ()))
            rebuilt.append(i)
        blk.instructions = rebuilt


def build_bass():
    """Build the per-core Bass program (identical on all 8 cores)."""
    _patch_tail_drain()
    nc = bass.Bass("TRN2", target_bir_lowering=False, debug=False)

    xt_d = nc.dram_tensor("xt", [E, S], BF16, kind="ExternalInput").ap()
    wq_d = nc.dram_tensor("wq", [E, EL], BF16, kind="ExternalInput").ap()
    wk_d = nc.dram_tensor("wk", [E, EL], BF16, kind="ExternalInput").ap()
    wv_d = nc.dram_tensor("wv", [E, EL], BF16, kind="ExternalInput").ap()
    wo_d = nc.dram_tensor("wo", [EL, E], BF16, kind="ExternalInput").ap()
    bq_d = nc.dram_tensor("bq2", [128, 2], F32, kind="ExternalInput").ap()
    out_d = nc.dram_tensor("out", [S, E], F32, kind="ExternalOutput").ap()

    EXP = mybir.ActivationFunctionType.Exp
    ADD = mybir.AluOpType.add
    MULT = mybir.AluOpType.mult

    with tile.TileContext(nc) as tc:
        with (
            tc.tile_pool(name="const", bufs=1) as const_pool,
            tc.tile_pool(name="xw", bufs=1) as xw_pool,
            tc.tile_pool(name="qkv", bufs=1) as qkv_pool,
            tc.tile_pool(name="exps", bufs=4) as exp_pool,
            tc.tile_pool(name="cn", bufs=3) as cn_pool,
            tc.tile_pool(name="dn", bufs=2) as dn_pool,
            tc.tile_pool(name="rr", bufs=2) as rr_pool,
            tc.tile_pool(name="rb", bufs=2) as rb_pool,
            tc.tile_pool(name="outs", bufs=4) as out_pool,
            tc.tile_pool(name="sc", bufs=2, space="PSUM") as sc_ps,
            tc.tile_pool(name="cx", bufs=4, space="PSUM") as ctx_ps,
        ):
            # ---- constants: one-hot lhsT tiles to broadcast recip row h4
            # (of a [4, ST] tile) to 64 output partitions via a K=4 matmul
            oh4 = []
            for h4 in range(HEADS_PER_CORE):
                oh = const_pool.tile([97, 64], FP16, name=f"oh{h4}")
                nc.vector.memset(oh[:], 0.0)
                nc.vector.memset(oh[32 * h4 : 32 * h4 + 1, :], 1.0)
                oh4.append(oh)
            bq_sb = const_pool.tile([128, 2], F32)
            nc.sync.dma_start(bq_sb[:], bq_d[:])

            # ---- weights + input DMA (bq first: tiny and needed early)
            xt_sb = xw_pool.tile([128, N_EC, S], BF16)
            for ec in range(N_EC):
                nc.sync.dma_start(xt_sb[:, ec, :], xt_d[128 * ec : 128 * (ec + 1), :])
            wq_sb = xw_pool.tile([128, N_EC, EL], BF16)
            nc.sync.dma_start(wq_sb[:], wq_d.rearrange("(o p) d -> p o d", p=128))
            wk_sb = xw_pool.tile([128, N_EC, EL], BF16)
            nc.sync.dma_start(wk_sb[:], wk_d.rearrange("(o p) d -> p o d", p=128))
            wv_sb = xw_pool.tile([128, N_EC, EL], BF16)
            nc.sync.dma_start(wv_sb[:], wv_d.rearrange("(o p) d -> p o d", p=128))
            wo_sb = xw_pool.tile([128, 2, E], BF16)
            nc.sync.dma_start(wo_sb[:], wo_d.rearrange("(o p) n -> p o n", p=128))

            # ---- persistent qkv storage
            qT = [qkv_pool.tile([128, S], BF16, name=f"qT{p}") for p in range(2)]
            kT = [qkv_pool.tile([128, S], BF16, name=f"kT{p}") for p in range(2)]
            # v with a fused all-ones column at [:, :, h, 64] (denominator row)
            v_sb = qkv_pool.tile([128, N_TC, HEADS_PER_CORE, DH + 1], BF16)
            nc.vector.memset(v_sb[:, :, :, DH : DH + 1], 1.0)

            def proj_q(st):
                slo, shi = ST * st, ST * (st + 1)
                for p in range(2):
                    dlo, dhi = 128 * p, 128 * (p + 1)
                    ps_q = sc_ps.tile([128, 2 * ST], F32, tag="sc")
                    for ec in range(N_EC):
                        nc.tensor.matmul(
                            ps_q[:, :ST],
                            wq_sb[:, ec, dlo:dhi],
                            xt_sb[:, ec, slo:shi],
                            start=(ec == 0),
                            stop=(ec == N_EC - 1),
                        )
                    nc.vector.tensor_scalar(
                        qT[p][:, slo:shi], ps_q[:, :ST], bq_sb[:, p : p + 1], None, ADD
                    )

            def proj_k(stk):
                slo, shi = ST * stk, ST * (stk + 1)
                for p in range(2):
                    dlo, dhi = 128 * p, 128 * (p + 1)
                    ps_k = sc_ps.tile([128, 2 * ST], F32, tag="sc")
                    for ec in range(N_EC):
                        nc.tensor.matmul(
                            ps_k[:, :ST],
                            wk_sb[:, ec, dlo:dhi],
                            xt_sb[:, ec, slo:shi],
                            start=(ec == 0),
                            stop=(ec == N_EC - 1),
                        )
                    nc.vector.tensor_copy(kT[p][:, slo:shi], ps_k[:, :ST])

            def proj_v(tt):
                ps_v = sc_ps.tile([128, 2 * ST], F32, tag="sc")
                for ec in range(N_EC):
                    nc.tensor.matmul(
                        ps_v[:, :EL],
                        xt_sb[:, ec, 128 * tt : 128 * (tt + 1)],
                        wv_sb[:, ec, :],
                        start=(ec == 0),
                        stop=(ec == N_EC - 1),
                    )
                # strided copy: [128, (4 heads @64)] -> [128, (4 heads @65)]
                nc.vector.tensor_copy(
                    v_sb[:, tt, :, 0:DH],
                    ps_v[:, :EL].rearrange("p (h d) -> p h d", h=HEADS_PER_CORE),
                )

            # ---- q(st0) and k chunk 0 first so exp can start ASAP
            proj_q(0)
            proj_k(0)

            # ---- attention + output projection, per s_tile
            for st in range(N_ST):
                slo, shi = ST * st, ST * (st + 1)
                ctx = [
                    [
                        ctx_ps.tile([128, ST], F32, tag="cx", name=f"ctx{p}{h}")
                        for h in range(2)
                    ]
                    for p in range(2)
                ]
                # rows {0,32,64,96} receive den; rest stay 1.0 so the
                # reciprocal feeding the one-hot broadcast matmul is finite
                dn = dn_pool.tile([97, ST], F32)
                nc.vector.memset(dn[:], 1.0)
                for tcn in range(N_TC):
                    if st == 0:
                        if tcn % 4 == 0 and tcn > 0:
                            proj_k(tcn // 4)
                        proj_v(tcn)
                    tlo, thi = 128 * tcn, 128 * (tcn + 1)
                    exs = []
                    for p in range(2):
                        sc = sc_ps.tile([128, 2 * ST], F32, tag="sc")
                        nc.tensor.matmul(
                            sc[:, :ST],
                            kT[p][0:64, tlo:thi],
                            qT[p][0:64, slo:shi],
                            start=True,
                            stop=True,
                        )
                        nc.tensor.matmul(
                            sc[:, ST:],
                            kT[p][64:128, tlo:thi],
                            qT[p][64:128, slo:shi],
                            start=True,
                            stop=True,
                        )
                        ex = exp_pool.tile([128, 2 * ST], BF16)
                        nc.scalar.activation(ex[:], sc[:], EXP, scale=0.125)
                        exs.append(ex)
                    first, last = tcn == 0, tcn == N_TC - 1
                    for p in range(2):
                        for h in range(2):
                            nc.tensor.matmul(
                                ctx[p][h][0 : DH + 1, :],
                                v_sb[:, tcn, 2 * p + h, :],
                                exs[p][:, ST * h : ST * (h + 1)],
                                start=first,
                                stop=last,
                            )

                # ---- normalize: gather 4 den rows, one batched recip,
                # ones-matmul broadcast, multiply.
                for p in range(2):
                    for h in range(2):
                        nc.vector.tensor_copy(
                            dn[32 * (2 * p + h) : 32 * (2 * p + h) + 1, :],
                            ctx[p][h][DH : DH + 1, :],
                        )
                rr = rr_pool.tile([97, ST], FP16)
                with nc.allow_low_precision(
                    reason="fp16 reciprocal rows: ~5e-4 rel err, under bf16 ctx"
                ):
                    nc.vector.reciprocal(rr[:], dn[:])

                # filler work for the tensor engine while recip runs on DVE
                if st + 1 < N_ST:
                    proj_q(st + 1)

                cns = []
                for p in range(2):
                    rbp = sc_ps.tile([128, 2 * ST], F32, tag="sc")
                    for h in range(2):
                        nc.tensor.matmul(
                            rbp[0:64, ST * h : ST * (h + 1)],
                            oh4[2 * p + h][:],
                            rr[:],
                            start=True,
                            stop=True,
                        )
                    rb = rb_pool.tile([64, 2 * ST], BF16)
                    nc.vector.tensor_copy(rb[:], rbp[0:64, :])
                    cn = cn_pool.tile([128, ST], BF16)
                    nc.vector.tensor_tensor(
                        cn[0:64, :], ctx[p][0][0:64, :], rb[:, :ST], MULT
                    )
                    nc.vector.tensor_tensor(
                        cn[64:128, :], ctx[p][1][0:64, :], rb[:, ST:], MULT
                    )
                    cns.append(cn)

                # ---- output projection for this s_tile
                for ss in range(ST // 128):
                    srow = slo + 128 * ss
                    for nt in range(E // ST):
                        ps_o = ctx_ps.tile([128, ST], F32, tag="cx")
                        nc.tensor.matmul(
                            ps_o[:],
                            cns[0][:, 128 * ss : 128 * (ss + 1)],
                            wo_sb[:, 0, ST * nt : ST * (nt + 1)],
                            start=True,
                            stop=False,
                        )
                        nc.tensor.matmul(
                            ps_o[:],
                            cns[1][:, 128 * ss : 128 * (ss + 1)],
                            wo_sb[:, 1, ST * nt : ST * (nt + 1)],
                            start=False,
                            stop=True,
                        )
                        ob = out_pool.tile([128, ST], F32)
                        nc.vector.tensor_copy(ob[:], ps_o[:])
                        nc.sync.dma_start(
                            out_d[srow : srow + 128, ST * nt : ST * (nt + 1)], ob[:]
                        )
    _split_multi_waits(nc)
    return nc


_NC = None


def _get_nc():
    global _NC
    if _NC is None:
        _NC = build_bass()
    return _NC


def make_in_maps(hidden_states, Wq, bq, Wk, Wv, Wo):
    """Host-side sharding/layout prep. Returns list of 8 per-core input dicts."""
    hs = np.asarray(hidden_states, dtype=np.float32)
    Wq = np.asarray(Wq, dtype=np.float32)
    Wk = np.asarray(Wk, dtype=np.float32)
    Wv = np.asarray(Wv, dtype=np.float32)
    Wo = np.asarray(Wo, dtype=np.float32)
    bq = np.asarray(bq, dtype=np.float32)

    xt = [
        np.ascontiguousarray(hs[b].T).astype(BF16_NP) for b in range(B)
    ]  # [E, S] bf16
    in_maps = []
    for c in range(N_CORES):
        b, g = divmod(c, N_CORES // B)
        h0 = HEADS_PER_CORE * g
        hsl = slice(h0, h0 + HEADS_PER_CORE)
        # [H_loc, E, DH] -> [E, H_loc*DH] head-major columns
        wq_c = np.ascontiguousarray(
            Wq[hsl].transpose(1, 0, 2).reshape(E, EL)
        ).astype(BF16_NP)
        wk_c = np.ascontiguousarray(
            Wk[hsl].transpose(1, 0, 2).reshape(E, EL)
        ).astype(BF16_NP)
        wv_c = np.ascontiguousarray(
            Wv[hsl].transpose(1, 0, 2).reshape(E, EL)
        ).astype(BF16_NP)
        wo_c = np.ascontiguousarray(Wo[EL * g : EL * (g + 1), :]).astype(BF16_NP)
        bq_c = np.ascontiguousarray(bq[hsl].reshape(EL).reshape(2, 128).T)
        in_maps.append(
            {
                "xt": xt[b],
                "wq": wq_c,
                "wk": wk_c,
                "wv": wv_c,
                "wo": wo_c,
                "bq2": bq_c,
            }
        )
    return in_maps


def kernel(hidden_states, mask, Wq, bq, Wk, bk, Wv, bv, Wo, bo, **run_kwargs):
    """Full-input entry point. mask is all-ones per the problem spec (ignored)."""
    nc = _get_nc()
    in_maps = make_in_maps(hidden_states, Wq, bq, Wk, Wv, Wo)
    res = run_bass_kernel_spmd(nc, in_maps, core_ids=list(range(N_CORES)), **run_kwargs)
    bo = np.asarray(bo, dtype=np.float32)
    bv = np.asarray(bv, dtype=np.float32)
    Wo_f = np.asarray(Wo, dtype=np.float32)
    # bv is a post-softmax additive constant: fold through the out projection
    bo_prime = bo + bv.reshape(E) @ Wo_f
    out = np.zeros((B, S, E), dtype=np.float32)
    for c in range(N_CORES):
        out[c // (N_CORES // B)] += res.results[c]["out"]
    out += bo_prime
    kernel.last_results = res
    return out


# revision 7
# speedup vs baseline: 1.2244x; 1.0696x over previous
"""Multi-head attention (B=2, S=2048, E=1024, H=16) on 8 Trainium2 NeuronCores.

Sharding: core c handles batch b=c//4 and head group g=c%4 (4 heads each).
hidden_states[b] is replicated to the 4 cores of batch b (pre-transposed and
cast to bf16 on host). Each core computes q/k/v projections for its heads,
transposed-layout attention (scores^T = k q'^T so softmax reduces over the
PSUM partition dim), and a partial output projection over its 256 E-dims.
The host sums the 4 partials per batch and adds bo'.

Structure: 8 "jobs" = (s_tile, head-pair). Each job runs a 16-chunk tc loop
(2 score matmuls -> exp -> 2 ctx matmuls per chunk). The Scalar engine's exp
stream is the kernel's critical resource (~810ns fixed + ~0.57ns/col per
ACTIVATE), so emission keeps it dense: projections are interleaved into the
first two jobs' tc loops (and st boundaries), and each job's softmax
normalization is DEFERRED into the next job's tc loop so no engine idles at
job boundaries. All transient PSUM (projection tiles, recip-broadcast,
output-projection) cycles through the scores pool ring; the 4 ctx banks hold
two jobs' accumulators (active + draining).

Softmax denominator: each head's v lhsT carries an extra all-ones column, so
the ctx matmul accumulates ctx rows 0:64 AND the denominator at row 64 of the
same PSUM bank for free. A job's two denominator rows are gathered to
partitions {0,32} of one tile, reciprocated in ONE vector op (recip is
~6.5ns/col regardless of partition count), broadcast to 64 partitions via
one-hot K=33 matmuls, and applied with tensor_tensor.

Bias/scale folding: softmax over t is invariant to per-query constants, so
the k-bias drops out and the q-bias is folded into q' = q + bq. The v-bias is
a post-softmax additive constant (softmax rows sum to 1) that commutes with
the output projection: host adds bo' = bo + concat(bv) @ Wo. The 1/sqrt(dh)
score scale is folded into Wk on host (exact: 0.125 is a power of two).
"""

import sys

if "/opt/trn_rl_repo" not in sys.path:
    sys.path.insert(0, "/opt/trn_rl_repo")

import numpy as np
import ml_dtypes

import concourse.bass as bass
import concourse.tile as tile
from concourse import mybir
from concourse.bass_utils import run_bass_kernel_spmd
from concourse.vector_clock import ScopedClock

B, S, E, H = 2, 2048, 1024, 16
DH = E // H  # 64
N_CORES = 8
HEADS_PER_CORE = 4
EL = HEADS_PER_CORE * DH  # 256 local E-dims per core

F32 = mybir.dt.float32
BF16 = mybir.dt.bfloat16
FP16 = mybir.dt.float16
BF16_NP = ml_dtypes.bfloat16

ST = 512  # s_tile width
N_ST = S // ST  # 4
N_TC = S // 128  # 16 t-chunks
N_EC = E // 128  # 8 e-chunks


def _patch_tail_drain():
    """walrus CoreV3 setupSyncWait allows only 1 sem wait on an SP Drain; Tile's
    kernel-tail drain carries one wait per live processor. Split the waits
    across consecutive drains (mutating via nc.inst_map, whose objects are what
    to_json_bytes serializes)."""
    if getattr(tile.TileContext, "_drain_patched", False):
        return

    def _drain_and_barrier(self, tick_clock, wait_clock):
        nc = self.nc
        drain_inst = nc.sync.drain()
        wait_clock.add_sem_waits(
            drain_inst.ins, ScopedClock({None: tick_clock.global_clock})
        )
        inst = nc.inst_map[drain_inst.ins.name]
        w = list(inst.sync_info.on_wait) if inst.sync_info else []
        if len(w) > 1:
            si = inst.sync_info
            si.on_wait = w[:1]
            inst.sync_info = si
            for i in range(1, len(w)):
                d2 = nc.sync.drain()
                i2 = nc.inst_map[d2.ins.name]
                si2 = i2.sync_info or mybir.SyncInfo(on_wait=[], on_update=[])
                si2.on_wait = [w[i]]
                i2.sync_info = si2
        nc.all_engine_barrier()
        assert self.sems is not None
        popped = nc._tile_sem_poison_stack.pop()
        assert popped is self._sem_poison
        nc.clear_and_free_semaphores(list(self.sems.allocated().values()))
        nc.all_engine_barrier()

    tile.TileContext._drain_and_barrier = _drain_and_barrier
    tile.TileContext._drain_patched = True


def _split_multi_waits(nc):
    """The walrus build in this environment accepts only ONE sem-wait command
    per instruction, but Tile's wait-assignment attaches several. Hoist excess
    waits onto dedicated same-engine no-op carrier instructions inserted
    immediately before the owner (same engine-stream position, identical
    semantics)."""
    f = nc.m.functions[0]
    blocks = list(f.blocks)
    carriers: dict[str, list] = {}
    created = set()
    for blk in blocks:
        for inst in blk.instructions:
            if inst.sync_info and len(inst.sync_info.on_wait) > 1:
                w = list(inst.sync_info.on_wait)
                cs = []
                for wx in w[:-1]:
                    # engine nop() appends to nc.cur_bb; it is re-homed below
                    nop = nc.engines[inst.engine].nop(nofuse=True).ins
                    nop.sync_info = mybir.SyncInfo(on_wait=[wx], on_update=[])
                    cs.append(nop)
                    created.add(nop.name)
                si = inst.sync_info
                si.on_wait = [w[-1]]
                inst.sync_info = si
                carriers[inst.name] = cs
    if not carriers:
        return
    for blk in blocks:
        rebuilt = []
        for i in blk.instructions:
            if i.name in created:
                continue
            rebuilt.extend(carriers.get(i.name, ()))
            rebuilt.append(i)
        blk.instructions = rebuilt


def build_bass():
    """Build the per-core Bass program (identical on all 8 cores)."""
    _patch_tail_drain()
    nc = bass.Bass("TRN2", target_bir_lowering=False, debug=False)

    xt_d = nc.dram_tensor("xt", [E, S], BF16, kind="ExternalInput").ap()
    wq_d = nc.dram_tensor("wq", [E, EL], BF16, kind="ExternalInput").ap()
    wk_d = nc.dram_tensor("wk", [E, EL], BF16, kind="ExternalInput").ap()
    wv_d = nc.dram_tensor("wv", [E, EL], BF16, kind="ExternalInput").ap()
    wo_d = nc.dram_tensor("wo", [EL, E], BF16, kind="ExternalInput").ap()
    bq_d = nc.dram_tensor("bq2", [128, 2], F32, kind="ExternalInput").ap()
    out_d = nc.dram_tensor("out", [S, E], F32, kind="ExternalOutput").ap()

    EXP = mybir.ActivationFunctionType.Exp
    ADD = mybir.AluOpType.add
    MULT = mybir.AluOpType.mult

    with tile.TileContext(nc) as tc:
        with (
            tc.tile_pool(name="const", bufs=1) as const_pool,
            tc.tile_pool(name="xw", bufs=1) as xw_pool,
            tc.tile_pool(name="qkv", bufs=1) as qkv_pool,
            tc.tile_pool(name="exps", bufs=4) as exp_pool,
            tc.tile_pool(name="cn", bufs=3) as cn_pool,
            tc.tile_pool(name="dn", bufs=2) as dn_pool,
            tc.tile_pool(name="rr", bufs=2) as rr_pool,
            tc.tile_pool(name="rb", bufs=2) as rb_pool,
            tc.tile_pool(name="outs", bufs=4) as out_pool,
            tc.tile_pool(name="sc", bufs=2, space="PSUM") as sc_ps,
            tc.tile_pool(name="cx", bufs=4, space="PSUM") as ctx_ps,
        ):
            # ---- constants: one-hot lhsT tiles broadcasting recip row 32h
            # (of a [33, ST] tile) to 64 output partitions via a K=33 matmul
            oh2 = []
            for h in range(2):
                oh = const_pool.tile([33, 64], FP16, name=f"oh{h}")
                nc.vector.memset(oh[:], 0.0)
                nc.vector.memset(oh[32 * h : 32 * h + 1, :], 1.0)
                oh2.append(oh)
            bq_sb = const_pool.tile([128, 2], F32)
            nc.sync.dma_start(bq_sb[:], bq_d[:])

            # ---- DMA: weights first (tiny), then xt split across the two
            # hardware DGE queues (SP + Activation) for parallel streaming
            wq_sb = xw_pool.tile([128, N_EC, EL], BF16)
            nc.sync.dma_start(wq_sb[:], wq_d.rearrange("(o p) d -> p o d", p=128))
            wk_sb = xw_pool.tile([128, N_EC, EL], BF16)
            nc.sync.dma_start(wk_sb[:], wk_d.rearrange("(o p) d -> p o d", p=128))
            wv_sb = xw_pool.tile([128, N_EC, EL], BF16)
            nc.sync.dma_start(wv_sb[:], wv_d.rearrange("(o p) d -> p o d", p=128))
            xt_sb = xw_pool.tile([128, N_EC, S], BF16)
            for ec in range(N_EC):
                eng = nc.sync if ec % 2 == 0 else nc.scalar
                eng.dma_start(xt_sb[:, ec, :], xt_d[128 * ec : 128 * (ec + 1), :])
            wo_sb = xw_pool.tile([128, 2, E], BF16)
            nc.scalar.dma_start(wo_sb[:], wo_d.rearrange("(o p) n -> p o n", p=128))

            # ---- persistent qkv storage
            qT = [qkv_pool.tile([128, S], BF16, name=f"qT{p}") for p in range(2)]
            kT = [qkv_pool.tile([128, S], BF16, name=f"kT{p}") for p in range(2)]
            # v with a fused all-ones column at [:, :, h, 64] (denominator row)
            v_sb = qkv_pool.tile([128, N_TC, HEADS_PER_CORE, DH + 1], BF16)
            nc.vector.memset(v_sb[:, :, :, DH : DH + 1], 1.0)

            def proj_qp(st, p):
                slo, shi = ST * st, ST * (st + 1)
                dlo, dhi = 128 * p, 128 * (p + 1)
                ps_q = sc_ps.tile([128, 2 * ST], F32, tag="sc", name="ps_q")
                for ec in range(N_EC):
                    nc.tensor.matmul(
                        ps_q[:, :ST],
                        wq_sb[:, ec, dlo:dhi],
                        xt_sb[:, ec, slo:shi],
                        start=(ec == 0),
                        stop=(ec == N_EC - 1),
                    )
                nc.vector.tensor_scalar(
                    qT[p][:, slo:shi], ps_q[:, :ST], bq_sb[:, p : p + 1], None, ADD
                )

            def proj_kp(p, stk):
                slo, shi = ST * stk, ST * (stk + 1)
                dlo, dhi = 128 * p, 128 * (p + 1)
                ps_k = sc_ps.tile([128, 2 * ST], F32, tag="sc", name="ps_k")
                for ec in range(N_EC):
                    nc.tensor.matmul(
                        ps_k[:, :ST],
                        wk_sb[:, ec, dlo:dhi],
                        xt_sb[:, ec, slo:shi],
                        start=(ec == 0),
                        stop=(ec == N_EC - 1),
                    )
                nc.vector.tensor_copy(kT[p][:, slo:shi], ps_k[:, :ST])

            def proj_v(tt):
                ps_v = sc_ps.tile([128, 2 * ST], F32, tag="sc", name="ps_v")
                for ec in range(N_EC):
                    nc.tensor.matmul(
                        ps_v[:, :EL],
                        xt_sb[:, ec, 128 * tt : 128 * (tt + 1)],
                        wv_sb[:, ec, :],
                        start=(ec == 0),
                        stop=(ec == N_EC - 1),
                    )
                nc.vector.tensor_copy(
                    v_sb[:, tt, :, 0:DH],
                    ps_v[:, :EL].rearrange("p (h d) -> p h d", h=HEADS_PER_CORE),
                )

            def emit_normalize(ctxp):
                """Deferred softmax-normalize of the previous job: gather den
                rows, one batched recip, one-hot broadcast matmuls, multiply.
                Returns the normalized [128, ST] bf16 cn tile."""
                dn = dn_pool.tile([33, ST], F32)
                for h in range(2):
                    nc.vector.tensor_copy(
                        dn[32 * h : 32 * h + 1, :], ctxp[h][DH : DH + 1, :]
                    )
                rr = rr_pool.tile([33, ST], FP16)
                with nc.allow_low_precision(
                    reason="fp16 reciprocal rows: ~5e-4 rel err, under bf16 ctx"
                ):
                    nc.vector.reciprocal(rr[:], dn[:])
                rbp = sc_ps.tile([128, 2 * ST], F32, tag="sc", name="rbp")
                for h in range(2):
                    nc.tensor.matmul(
                        rbp[0:64, ST * h : ST * (h + 1)],
                        oh2[h][:],
                        rr[:],
                        start=True,
                        stop=True,
                    )
                rb = rb_pool.tile([64, 2 * ST], BF16)
                nc.vector.tensor_copy(rb[:], rbp[0:64, :])
                cn = cn_pool.tile([128, ST], BF16)
                nc.vector.tensor_tensor(cn[0:64, :], ctxp[0][0:64, :], rb[:, :ST], MULT)
                nc.vector.tensor_tensor(
                    cn[64:128, :], ctxp[1][0:64, :], rb[:, ST:], MULT
                )
                return cn

            def emit_outproj_chunk(st, cns, ss, nt):
                srow = ST * st + 128 * ss
                ps_o = sc_ps.tile([128, 2 * ST], F32, tag="sc", name="ps_o")
                nc.tensor.matmul(
                    ps_o[:, :ST],
                    cns[0][:, 128 * ss : 128 * (ss + 1)],
                    wo_sb[:, 0, ST * nt : ST * (nt + 1)],
                    start=True,
                    stop=False,
                )
                nc.tensor.matmul(
                    ps_o[:, :ST],
                    cns[1][:, 128 * ss : 128 * (ss + 1)],
                    wo_sb[:, 1, ST * nt : ST * (nt + 1)],
                    start=False,
                    stop=True,
                )
                ob = out_pool.tile([128, ST], F32)
                nc.vector.tensor_copy(ob[:], ps_o[:, :ST])
                nc.sync.dma_start(
                    out_d[srow : srow + 128, ST * nt : ST * (nt + 1)], ob[:]
                )

            # ---- preamble: first q tile + first k chunk so exp starts ASAP
            proj_qp(0, 0)
            proj_kp(0, 0)

            pending = None  # ctx pair awaiting deferred normalize
            cns_by_st = {}
            for j in range(2 * N_ST):
                st, p = j // 2, j % 2
                slo, shi = ST * st, ST * (st + 1)
                ctxp = [
                    ctx_ps.tile([128, ST], F32, tag="cx", name=f"ctx{j}_{h}")
                    for h in range(2)
                ]
                for tcn in range(N_TC):
                    # scheduled projection / drain work interleaved into the loop
                    if j == 0:
                        proj_v(tcn)
                        if tcn in (4, 8, 12):
                            proj_kp(0, tcn // 4)
                        if tcn == 14:
                            proj_kp(1, 0)
                    if j == 1 and tcn in (1, 5, 9):
                        proj_kp(1, tcn // 4 + 1)
                    if tcn == 12:
                        if p == 0:
                            proj_qp(st, 1)
                        elif st + 1 < N_ST:
                            proj_qp(st + 1, 0)
                    if tcn == 3 and pending is not None:
                        jp, ctx_prev = pending
                        cns_by_st.setdefault(jp // 2, {})[jp % 2] = emit_normalize(
                            ctx_prev
                        )
                        pending = None
                    if p == 0 and st >= 1 and 4 <= tcn < 12:
                        i = tcn - 4
                        cns = cns_by_st[st - 1]
                        emit_outproj_chunk(st - 1, [cns[0], cns[1]], i // 2, i % 2)

                    # core attention step
                    tlo, thi = 128 * tcn, 128 * (tcn + 1)
                    sc = sc_ps.tile([128, 2 * ST], F32, tag="sc", name="sc")
                    nc.tensor.matmul(
                        sc[:, :ST],
                        kT[p][0:64, tlo:thi],
                        qT[p][0:64, slo:shi],
                        start=True,
                        stop=True,
                    )
                    nc.tensor.matmul(
                        sc[:, ST:],
                        kT[p][64:128, tlo:thi],
                        qT[p][64:128, slo:shi],
                        start=True,
                        stop=True,
                    )
                    ex = exp_pool.tile([128, 2 * ST], BF16)
                    nc.scalar.activation(ex[:], sc[:], EXP, scale=1.0)
                    first, last = tcn == 0, tcn == N_TC - 1
                    for h in range(2):
                        nc.tensor.matmul(
                            ctxp[h][0 : DH + 1, :],
                            v_sb[:, tcn, 2 * p + h, :],
                            ex[:, ST * h : ST * (h + 1)],
                            start=first,
                            stop=last,
                        )
                pending = (j, ctxp)

            # ---- tail: last job's normalize + last s_tile's output projection
            jp, ctx_prev = pending
            cns_by_st.setdefault(jp // 2, {})[jp % 2] = emit_normalize(ctx_prev)
            cns = cns_by_st[N_ST - 1]
            for i in range(8):
                emit_outproj_chunk(N_ST - 1, [cns[0], cns[1]], i // 2, i % 2)
    _split_multi_waits(nc)
    return nc


_NC = None


def _get_nc():
    global _NC
    if _NC is None:
        _NC = build_bass()
    return _NC


def make_in_maps(hidden_states, Wq, bq, Wk, Wv, Wo):
    """Host-side sharding/layout prep. Returns list of 8 per-core input dicts."""
    hs = np.asarray(hidden_states, dtype=np.float32)
    Wq = np.asarray(Wq, dtype=np.float32)
    Wk = np.asarray(Wk, dtype=np.float32) * 0.125  # fold 1/sqrt(DH) into Wk
    Wv = np.asarray(Wv, dtype=np.float32)
    Wo = np.asarray(Wo, dtype=np.float32)
    bq = np.asarray(bq, dtype=np.float32)

    xt = [
        np.ascontiguousarray(hs[b].T).astype(BF16_NP) for b in range(B)
    ]  # [E, S] bf16
    in_maps = []
    for c in range(N_CORES):
        b, g = divmod(c, N_CORES // B)
        h0 = HEADS_PER_CORE * g
        hsl = slice(h0, h0 + HEADS_PER_CORE)
        # [H_loc, E, DH] -> [E, H_loc*DH] head-major columns
        wq_c = np.ascontiguousarray(
            Wq[hsl].transpose(1, 0, 2).reshape(E, EL)
        ).astype(BF16_NP)
        wk_c = np.ascontiguousarray(
            Wk[hsl].transpose(1, 0, 2).reshape(E, EL)
        ).astype(BF16_NP)
        wv_c = np.ascontiguousarray(
            Wv[hsl].transpose(1, 0, 2).reshape(E, EL)
        ).astype(BF16_NP)
        wo_c = np.ascontiguousarray(Wo[EL * g : EL * (g + 1), :]).astype(BF16_NP)
        bq_c = np.ascontiguousarray(bq[hsl].reshape(EL).reshape(2, 128).T)
        in_maps.append(
            {
                "xt": xt[b],
                "wq": wq_c,
                "wk": wk_c,
                "wv": wv_c,
                "wo": wo_c,
                "bq2": bq_c,
            }
        )
    return in_maps


def kernel(hidden_states, mask, Wq, bq, Wk, bk, Wv, bv, Wo, bo, **run_kwargs):
    """Full-input entry point. mask is all-ones per the problem spec (ignored)."""
    nc = _get_nc()
    in_maps = make_in_maps(hidden_states, Wq, bq, Wk, Wv, Wo)
    res = run_bass_kernel_spmd(nc, in_maps, core_ids=list(range(N_CORES)), **run_kwargs)
    bo = np.asarray(bo, dtype=np.float32)
    bv = np.asarray(bv, dtype=np.float32)
    Wo_f = np.asarray(Wo, dtype=np.float32)
    # bv is a post-softmax additive constant: fold through the out projection
    bo_prime = bo + bv.reshape(E) @ Wo_f
    out = np.zeros((B, S, E), dtype=np.float32)
    for c in range(N_CORES):
        out[c // (N_CORES // B)] += res.results[c]["out"]
    out += bo_prime
    kernel.last_results = res
    return out
